# revision 64
# baseline (speedup 1.0000x reference)
"""Causal self-attention (GQA, rope) on 8 Trainium2 NeuronCores.

Sharding: tensor-parallel over the 4 kv-head groups x data-parallel over the
batch of 2.  Core c handles batch b = c // 4 and kv-group g = c % 4:

  - q/k/v projections for the group's 4 q-heads + 1 kv-head,
  - rope, causal flash-style attention (unnormalized softmax: e = exp(s),
    y = (e @ v) * (1 / (e @ 1)) -- safe here because scores are O(1)),
  - partial output projection out_partial = y_g @ wproj[:, cols_g].T.

The host sums the 4 group partials per batch element.

All matmuls run in bf16 with fp32 PSUM accumulation.  Activations are cast
host-side; x / weights are pre-transposed host-side so the contraction dim
lands on SBUF partitions without any on-device fp32 transposes.

Schedule notes (from perfetto/NTFF trace iteration, 287us -> ~252us):
  - phase B interleaves the two 8-bank q cohorts with the k/v projection
    pairs so each cohort's rope drain (~1.5us/tile on ACT+DVE) overlaps
    ~14us of k/v matmuls instead of stalling the PE;
  - rope epilogue: one ACT bias-copy frees the psum bank early, two bf16
    2x-mode DVE muls, one psum-sourced stt (up-shifted SBUF reads run at
    <1/2 rate), final add on GpSimd (q) or DVE (k -- attention needs kT
    early and GpSimd is a strict FIFO);
  - DMA: ~600ns issue/descriptor, 8 global completion lanes of ~2.4us
    each; both HWDGE queues (Sync: x, Scalar: weights) issue in strict
    need-order, small tensors packed into single descriptors;
  - phase C streams (score, exp, e@v) blocks with a 2-block lookahead;
    per-row den/recip/broadcast/norm + the previous row's projection are
    interleaved; row order ends on the 1-key-tile row to shorten the tail.
"""

import numpy as np
import ml_dtypes

BF16 = ml_dtypes.bfloat16

T = 2048
C = 2048
HD = 128
N_KV = 4
N_REP = 4
O_G = N_REP * HD  # 512 q-dims per group
TC = 512  # t-chunk (psum bank width in fp32)
N_TC = T // TC  # 4
N_KT = C // 128  # 16 contraction tiles
SCALE = float(1.0 / np.sqrt(HD))

_compiled = None


def _build():
    import concourse.bacc as bacc
    import concourse.mybir as mybir
    import concourse.tile as tile
    from concourse import bass_isa
    from concourse.masks import make_identity

    f32 = mybir.dt.float32
    bf16 = mybir.dt.bfloat16

    nc = bacc.Bacc("TRN2", target_bir_lowering=False, debug=False)

    # small inputs are PACKED into single tensors: the Sync queue issues DMA
    # descriptors at only ~1.5/us, so descriptor COUNT (not bytes) gates how
    # soon late-queue tensors land
    xT = nc.dram_tensor("xT", [C, T], bf16, kind="ExternalInput").ap()
    wqT = nc.dram_tensor("wqT", [C, O_G], bf16, kind="ExternalInput").ap()
    wkvT = nc.dram_tensor("wkvT", [C, 2 * HD], bf16, kind="ExternalInput").ap()
    wpT = nc.dram_tensor("wpT", [O_G, C], bf16, kind="ExternalInput").ap()
    # biases: cols 0:4 bq, 4:8 bqs (half-swapped), 8 bk, 9 bks, 10 bv
    biases = nc.dram_tensor("biases", [HD, 11], f32, kind="ExternalInput").ap()
    # ropes: [A | B | C] along the free dim
    ropes = nc.dram_tensor("ropes", [HD, 3 * T], bf16, kind="ExternalInput").ap()
    masks = nc.dram_tensor("masks", [128, N_REP, 128], bf16, kind="ExternalInput").ap()
    out = nc.dram_tensor("out", [T, C], bf16, kind="ExternalOutput").ap()

    Exp = mybir.ActivationFunctionType.Exp
    Copy = mybir.ActivationFunctionType.Copy

    with tile.TileContext(nc) as tc:
        import contextlib

        with contextlib.ExitStack() as ctx:
            persist = ctx.enter_context(tc.tile_pool(name="persist", bufs=1))

            # ---- persistent SBUF tensors ----
            wpT_sb = persist.tile([128, N_REP, C], bf16)
            qT_sb = persist.tile([128, N_REP, T], bf16)
            kT_sb = persist.tile([128, T], bf16)
            v_sb = persist.tile([128, N_KT, HD], bf16)
            yT_sb = persist.tile([128, N_REP, T], bf16)
            masks_sb = persist.tile([128, N_REP, 128], bf16)
            onescol = persist.tile([128, 1], bf16)
            onesrow = persist.tile([1, 128], bf16)
            ident = persist.tile([128, 128], bf16)

            # ---- phase B (projections): scoped pools ----
            bctx = contextlib.ExitStack()
            bpool = bctx.enter_context(tc.tile_pool(name="phase_b", bufs=1))
            # bufs=8: the ACT bias-copy of rope i recycles the u tile of
            # rope i-bufs, so shallow buffering gates psum-bank frees on the
            # SLOWEST rope op (the trailing GpSimd ops, ~2us/rope); 8 bufs
            # covers a full cohort so ACT (and the banks) free-run
            tmp_pool = bctx.enter_context(tc.tile_pool(name="rope_tmp", bufs=8))
            xT_sb = bpool.tile([128, N_KT, T], bf16)
            wqT_sb = bpool.tile([128, N_KT, O_G], bf16)
            wkvT_sb = bpool.tile([128, N_KT, 2 * HD], bf16)
            ropes_sb = bpool.tile([128, 3, T], bf16)
            biases_sb = bpool.tile([HD, 11], f32)
            vT_sb = bpool.tile([128, T], bf16)
            ropeA_sb = ropes_sb[:, 0, :]
            ropeB_sb = ropes_sb[:, 1, :]
            ropeC_sb = ropes_sb[:, 2, :]
            bq_sb = biases_sb[:, 0:N_REP]
            bqs_sb = biases_sb[:, N_REP : 2 * N_REP]
            bk_sb = biases_sb[:, 8:9]
            bks_sb = biases_sb[:, 9:10]
            bv_sb = biases_sb[:, 10:11]

            nc.vector.memset(onescol[:], 1.0)
            nc.vector.memset(onesrow[:], 1.0)
            make_identity(nc, ident[:])

            # dummy matmuls while the first DMAs land: keeps the PE busy so the
            # HAM clock-gate is already at 8/8 when real work starts
            with tc.tile_pool(name="warmpsum", bufs=1, space="PSUM") as warmpool:
                wps = warmpool.tile([128, 128], f32)
                for _ in range(20):
                    nc.tensor.matmul(wps[:], lhsT=ident[:], rhs=ident[:],
                                     start=True, stop=True)

            # per-contraction-tile loads, in consumption order, so the first
            # projection matmuls start after ~384KB instead of ~13MB.  The
            # first q sweep (tcg=0) only touches x columns 0:1024, so x tiles
            # are split in half: front halves first (tcg=0's exact working
            # set, 6.3MB over its 27us of PE work), back halves after.
            xT_r = xT.rearrange("(kt p) t -> p kt t", p=128)
            wqT_r = wqT.rearrange("(kt p) o -> p kt o", p=128)
            ropes_r = ropes.rearrange("p (r t) -> p r t", r=3)
            HT = T // 2
            QT = T // 4
            # DMA model: ~600ns issue per descriptor on the issuing HWDGE
            # queue, 8 global completion lanes each holding ONE in-flight
            # DMA through transfer + ~2us HBM-receipt.  So (a) use both
            # HWDGE queues (Sync: x, Scalar: weights/tables) and (b) issue
            # in STRICT need-order -- a 1.5MB table mid-queue occupies a
            # lane for ~6us and starves every descriptor behind it.
            for kt in range(N_KT):
                if kt < 2:
                    # the very first tiles gate the first real matmuls; split
                    # them across lanes so the critical chunks land sooner
                    nc.scalar.dma_start(
                        wqT_sb[:, kt, 0:256], wqT_r[:, kt, 0:256]
                    )
                    nc.scalar.dma_start(
                        wqT_sb[:, kt, 256:512], wqT_r[:, kt, 256:512]
                    )
                    nc.sync.dma_start(xT_sb[:, kt, 0:QT], xT_r[:, kt, 0:QT])
                    nc.sync.dma_start(
                        xT_sb[:, kt, QT:HT], xT_r[:, kt, QT:HT]
                    )
                else:
                    nc.scalar.dma_start(wqT_sb[:, kt, :], wqT_r[:, kt, :])
                    nc.sync.dma_start(xT_sb[:, kt, 0:HT], xT_r[:, kt, 0:HT])
                if kt == 0:
                    nc.scalar.dma_start(biases_sb[:], biases[:])
                if kt == 13:
                    # wkv is needed by kv_pair(0,1) right after tcg0 (~42us).
                    # At the back of the queue it lands ~47us (4.1us PE
                    # stall); at kt==9 it delays wq10-12 past their use
                    # (4.8us stall).  Split halves near the stream tail,
                    # one per queue, dodge both.
                    wkvT_r = wkvT.rearrange("(kt p) o -> p kt o", p=128)
                    nc.scalar.dma_start(
                        wkvT_sb[:, 0:8, :], wkvT_r[:, 0:8, :]
                    )
            nc.sync.dma_start(wkvT_sb[:, 8:16, :], wkvT_r[:, 8:16, :])
            # everything below is needed >=15us after the above finish
            nc.scalar.dma_start(ropes_sb[:], ropes_r[:])
            # x back halves (for tcg1, ~53us+), 4 kt-tiles per descriptor
            for ktq in range(N_KT // 4):
                nc.sync.dma_start(
                    xT_sb[:, 4 * ktq : 4 * ktq + 4, HT:T],
                    xT_r[:, 4 * ktq : 4 * ktq + 4, HT:T],
                )
            nc.scalar.dma_start(masks_sb[:], masks[:])
            nc.scalar.dma_start(
                wpT_sb[:], wpT.rearrange("(h p) m -> p h m", p=128)
            )

            Identity = mybir.ActivationFunctionType.Identity

            def rope_epilogue(ps, dst, t0, bias, bias_sw, add_on_dve=False):
                """dst (bf16 SBUF) = rope(ps + bias) using A/B/C tables; ps is
                fp32 psum [128, TC] at positions t0:t0+TC, bias a [128,1]
                per-partition column.  One ACT op folds the bias and frees
                the psum bank for reuse after two readers (u = ps + b); the
                down-shifted cross mul and the A mul run bf16-on-SBUF where
                DVE hits 2x mode (~420ns); the UP-shifted cross mul reads
                the psum directly (fp32 stt, ~720ns) because an SBUF 16-bit
                read shifted up 64 partitions runs at less than half rate
                (measured 1050ns vs 420ns).  The final add goes to GpSimd
                for q (16 tiles, keeps DVE free) but to DVE for k so the
                attention stream's kT tiles don't queue behind all the q
                adds in GpSimd's strict FIFO."""
                A = ropeA_sb[:, t0 : t0 + TC]
                Bm = ropeB_sb[:, t0 : t0 + TC]
                Cm = ropeC_sb[:, t0 : t0 + TC]
                u = tmp_pool.tile([128, TC], bf16, tag="rope_u")
                tmp = tmp_pool.tile([128, TC], bf16, tag="rope_tmp")
                tmp2 = tmp_pool.tile([128, TC], bf16, tag="rope_tmp2")
                mult = mybir.AluOpType.mult
                add = mybir.AluOpType.add
                nc.scalar.activation(u[:], ps[:], Identity, bias=bias)
                # ropeB rows 64:128 hold -sin so u and B read the same base
                # partition (walrus requires equal bases for two SBUF inputs)
                nc.vector.tensor_mul(tmp[0:64, :], u[64:128, :], Bm[64:128, :])
                nc.vector.scalar_tensor_tensor(
                    tmp[64:128, :], ps[0:64, :], bias_sw[64:128, :],
                    Cm[64:128, :], add, mult,
                )
                nc.vector.tensor_mul(tmp2[:], u[:], A)
                if add_on_dve:
                    nc.vector.tensor_add(dst, tmp2[:], tmp[:])
                else:
                    nc.gpsimd.tensor_add(dst, tmp2[:], tmp[:])

            # ---- phase B: q/k/v projections ----
            # q proj runs kt-OUTER over groups of 2 t-chunks (8 psum banks
            # live), so each 640KB (x,wq) kt-tile is consumed right after its
            # DMA lands instead of the first psum tile needing all 10MB.
            # ONE psum pool for all of phase B: q tiles, k/v tiles and v
            # transposes share the same 8-buffer rotation, so the k matmuls
            # slot into banks as the q ropes free them (a separate pool would
            # barrier on ALL pending ropes -- measured 19.8us stall).
            with tc.tile_pool(name="bpsum", bufs=8, space="PSUM") as bpsum:

                def q_group(tcg):
                    pss = []
                    for tci in (2 * tcg, 2 * tcg + 1):
                        for ot in range(N_REP):
                            pss.append(
                                (
                                    tci,
                                    ot,
                                    bpsum.tile(
                                        [128, TC], f32, tag="b",
                                        name=f"ps_q{tci}_{ot}",
                                    ),
                                )
                            )
                    for kt in range(N_KT):
                        for tci, ot, ps in pss:
                            nc.tensor.matmul(
                                ps[:],
                                lhsT=wqT_sb[:, kt, ot * 128 : (ot + 1) * 128],
                                rhs=xT_sb[:, kt, tci * TC : (tci + 1) * TC],
                                start=(kt == 0),
                                stop=(kt == N_KT - 1),
                            )
                    for tci, ot, ps in pss:
                        rope_epilogue(
                            ps, qT_sb[:, ot, tci * TC : (tci + 1) * TC],
                            tci * TC,
                            bq_sb[:, ot : ot + 1], bqs_sb[:, ot : ot + 1],
                        )

                def kv_pair(tciA, tciB):
                    """k+v projections for two t-chunks, then ALL their v
                    transposes.  The transposes' LDWEIGHTS need vT (the
                    bias-cast output), and that cast queues behind the q
                    cohort's rope drain on its engine FIFO -- putting ~14us
                    of projection matmuls before the first transpose hides
                    the whole drain (measured 7-8us PE stall otherwise)."""
                    for tci in (tciA, tciB):
                        t0 = tci * TC
                        psk = bpsum.tile([128, TC], f32, tag="b", name="ps_k")
                        for kt in range(N_KT):
                            nc.tensor.matmul(
                                psk[:],
                                lhsT=wkvT_sb[:, kt, 0:HD],
                                rhs=xT_sb[:, kt, t0 : t0 + TC],
                                start=(kt == 0),
                                stop=(kt == N_KT - 1),
                            )
                        rope_epilogue(
                            psk, kT_sb[:, t0 : t0 + TC], t0,
                            bk_sb[:], bks_sb[:], add_on_dve=True,
                        )

                        psv = bpsum.tile([128, TC], f32, tag="b", name="ps_v")
                        for kt in range(N_KT):
                            nc.tensor.matmul(
                                psv[:],
                                lhsT=wkvT_sb[:, kt, HD : 2 * HD],
                                rhs=xT_sb[:, kt, t0 : t0 + TC],
                                start=(kt == 0),
                                stop=(kt == N_KT - 1),
                            )
                        # v bias folded into the psum->sbuf cast
                        nc.scalar.activation(
                            vT_sb[:, t0 : t0 + TC], psv[:], Identity,
                            bias=bv_sb[:],
                        )

                    # transpose both chunks' vT -> v (natural [t, d]) via PE
                    for tci in (tciA, tciB):
                        for jt in range(4 * tci, 4 * tci + 4):
                            pst = bpsum.tile(
                                [128, TC], bf16, tag="b", name="ps_t"
                            )
                            nc.tensor.transpose(
                                pst[:, 0:128],
                                vT_sb[:, jt * 128 : (jt + 1) * 128], ident[:],
                            )
                            nc.scalar.copy(v_sb[:, jt, :], pst[:, 0:128])

                # interleave: each q cohort's 8 psum tiles all complete at its
                # sweep end, then ~1.5us/tile of rope drain must pass before
                # the banks free.  The kv pairs (which only need 1-2 banks,
                # available after the first rope) give the PE ~15us of work
                # to overlap each drain instead of idling on it.
                q_group(0)
                kv_pair(0, 1)
                q_group(1)
                kv_pair(2, 3)

            bctx.close()

            # ---- phase C (attention) ----
            # Packed-head stream: each matmul handles all 4 q-heads x 128
            # queries (N=512).  GQA means the 4 heads share this group's
            # kv head, so k/v lhsT tiles are head-independent and causal
            # granularity drops to 128 queries (53% of T^2 vs 62.5%).
            # Rows (ci, qs) normalize + output-project per row (1-row lag).
            stage_pool = ctx.enter_context(tc.tile_pool(name="stage", bufs=4))
            epool = ctx.enter_context(tc.tile_pool(name="e", bufs=3))
            esum_pool = ctx.enter_context(tc.tile_pool(name="esum", bufs=2))
            dci_pool = ctx.enter_context(tc.tile_pool(name="dci", bufs=2))

            with (
                tc.tile_pool(name="spsum", bufs=2, space="PSUM") as spsum,
                tc.tile_pool(name="ypsum", bufs=3, space="PSUM") as ypsum,
                tc.tile_pool(name="dpsum", bufs=1, space="PSUM") as dpsum,
            ):

                def s_group(qi, g):
                    """Score matmuls for key tiles 2g(,2g+1) against the
                    packed [4 heads x 128 queries] block qi."""
                    n_sub = min(2, (qi + 1) - 2 * g)
                    ps = spsum.tile([128, 2, TC], f32, tag="s")
                    for sub in range(n_sub):
                        jt = 2 * g + sub
                        nc.tensor.matmul(
                            ps[:, sub, :],
                            lhsT=kT_sb[:, jt * 128 : (jt + 1) * 128],
                            rhs=qT_sb[:, :, qi * 128 : (qi + 1) * 128],
                            start=True,
                            stop=True,
                        )
                    return ps

                def emit_proj(qi, n_chunks=2):
                    """Output projection for t-tile qi (its row's yT is
                    normalized one row earlier).  The 512KB row store is
                    split into chunks so it spreads across DMA queues; the
                    final row uses 4 chunks since its store is the kernel
                    tail."""
                    o_sb = stage_pool.tile([128, N_TC, TC], bf16, tag="o_stage")
                    for mc in range(N_TC):
                        ps_o = ypsum.tile([128, TC], f32, tag="y")
                        for h in range(N_REP):
                            nc.tensor.matmul(
                                ps_o[:],
                                lhsT=yT_sb[:, h, qi * 128 : (qi + 1) * 128],
                                rhs=wpT_sb[:, h, mc * TC : (mc + 1) * TC],
                                start=(h == 0),
                                stop=(h == N_REP - 1),
                            )
                        if mc % 2 == 0:
                            nc.scalar.copy(o_sb[:, mc, :], ps_o[:])
                        else:
                            nc.vector.tensor_copy(o_sb[:, mc, :], ps_o[:])
                    row = out[qi * 128 : (qi + 1) * 128, :]
                    mcs = N_TC // n_chunks
                    for ch in range(n_chunks):
                        nc.sync.dma_start(
                            row[:, ch * mcs * TC : (ch + 1) * mcs * TC],
                            o_sb[:, ch * mcs : (ch + 1) * mcs, :],
                        )

                # flat stream of (qi, group) with a 2-deep score-matmul
                # lookahead crossing row boundaries so the PE never drains
                all_blocks = []
                # end on row 0: the final row's serial den/recip/norm/proj
                # chain is the kernel tail, so give it the shortest row
                row_order = list(range(1, 12)) + [15, 14, 13, 12, 0]
                for qi in row_order:
                    for g in range((qi + 2) // 2):
                        all_blocks.append((qi, g))
                nblk = len(all_blocks)
                s_tiles = {}

                def emit_s(b):
                    if b >= nblk:
                        return
                    qi, g = all_blocks[b]
                    s_tiles[b] = s_group(qi, g)

                emit_s(0)
                emit_s(1)
                state = {}
                for b, (qi, g) in enumerate(all_blocks):
                    n_tiles = qi + 1
                    ng = (n_tiles + 1) // 2
                    n_sub = min(2, n_tiles - 2 * g)
                    if g == 0:
                        state["ps_y"] = ypsum.tile(
                            [128, TC], f32, tag="y", name="ps_y"
                        )
                        state["esum"] = esum_pool.tile(
                            [128, 2, TC], bf16, tag="esum", name="esum"
                        )
                    ps_s = s_tiles.pop(b)
                    e = epool.tile([128, 2, TC], bf16)
                    nc.scalar.activation(
                        e[:, 0:n_sub, :], ps_s[:, 0:n_sub, :], Exp, scale=SCALE
                    )
                    if g == (n_tiles - 1) // 2:
                        # group holding the diagonal key tile: triangular mask
                        ds = (n_tiles - 1) % 2
                        nc.vector.tensor_mul(
                            e[:, ds, :], e[:, ds, :], masks_sb[:]
                        )
                    # denominator: accumulate e on DVE; PE row-sums once/row
                    if g == 0:
                        nc.vector.tensor_copy(
                            state["esum"][:, 0:n_sub, :], e[:, 0:n_sub, :]
                        )
                    else:
                        nc.vector.tensor_add(
                            state["esum"][:, 0:n_sub, :],
                            state["esum"][:, 0:n_sub, :],
                            e[:, 0:n_sub, :],
                        )
                    for sub in range(n_sub):
                        jt = 2 * g + sub
                        nc.tensor.matmul(
                            state["ps_y"][:],
                            lhsT=v_sb[:, jt, :],
                            rhs=e[:, sub, :],
                            start=(jt == 0),
                            stop=(jt == n_tiles - 1),
                        )
                    emit_s(b + 2)
                    if g == ng - 1:
                        # row complete: den row-sum -> 1/den -> broadcast ->
                        # normalize this row's yT in place
                        ps_d = dpsum.tile([128, TC], f32, tag="d", name="ps_d")
                        n_den = 1 if n_tiles == 1 else 2
                        for sub in range(n_den):
                            nc.tensor.matmul(
                                ps_d[0:1, :],
                                lhsT=onescol[:],
                                rhs=state["esum"][:, sub, :],
                                start=(sub == 0),
                                stop=(sub == n_den - 1),
                            )
                        rden_f = dci_pool.tile([1, TC], f32, tag="rdenf")
                        nc.vector.reciprocal_approx_fast(
                            rden_f[:], ps_d[0:1, :]
                        )
                        rden_bf = dci_pool.tile([1, TC], bf16, tag="rden")
                        nc.vector.tensor_copy(rden_bf[:], rden_f[:])
                        # broadcast 1/den to 128 partitions via PE (same bank)
                        nc.tensor.matmul(
                            ps_d[:],
                            lhsT=onesrow[:],
                            rhs=rden_bf[:],
                            start=True,
                            stop=True,
                        )
                        rb_sb = stage_pool.tile([128, TC], f32, tag="rb_stage")
                        nc.vector.tensor_copy(rb_sb[:], ps_d[:])
                        nc.vector.tensor_mul(
                            yT_sb[:, :, qi * 128 : (qi + 1) * 128],
                            state["ps_y"][:],
                            rb_sb[:],
                        )
                        if state.get("prev_row") is not None:
                            emit_proj(state["prev_row"])
                        state["prev_row"] = qi
                emit_proj(state["prev_row"], n_chunks=4)

    nc.compile()
    return nc


def _get_compiled():
    global _compiled
    if _compiled is None:
        _compiled = _build()
    return _compiled


def kernel(x, cos, sin, wq, bq, wk, bk, wv, bv, wproj):
    from concourse.bass_utils import run_bass_kernel_spmd

    nc = _get_compiled()

    x = np.asarray(x, np.float32)
    wq = np.asarray(wq, np.float32)
    bq = np.asarray(bq, np.float32)
    wk = np.asarray(wk, np.float32)
    bk = np.asarray(bk, np.float32)
    wv = np.asarray(wv, np.float32)
    bv = np.asarray(bv, np.float32)
    wproj = np.asarray(wproj, np.float32)
    cosT = np.asarray(cos, np.float32)[0, :, 0, :].T  # (64, T)
    sinT = np.asarray(sin, np.float32)[0, :, 0, :].T
    ropeA = np.concatenate([cosT, cosT], 0)  # (128, T)
    # B: rows 64:128 = -sin (read with u[64:128] for the low cross term);
    # C: rows 64:128 = +sin (read by the up-shifted stt); low halves unused
    ropeB = np.concatenate([sinT, -sinT], 0)
    ropeC = np.concatenate([-sinT, sinT], 0)
    ropes = np.ascontiguousarray(
        np.concatenate([ropeA, ropeB, ropeC], 1)
    ).astype(BF16)  # (128, 3T)

    jj = np.arange(128, dtype=np.int64)[:, None, None]
    ii = np.arange(128, dtype=np.int64)[None, None, :]
    masks = np.ascontiguousarray(
        np.broadcast_to(jj <= ii, (128, N_REP, 128))
    ).astype(BF16)  # triangular tile, replicated per head

    xT = [np.ascontiguousarray(x[b].T).astype(BF16) for b in range(2)]

    bq_t = [
        np.ascontiguousarray(
            bq[g * O_G : (g + 1) * O_G].reshape(N_REP, HD).T
        ).astype(np.float32)
        for g in range(4)
    ]
    bk_t = [
        bk[g * HD : (g + 1) * HD, None].astype(np.float32) for g in range(4)
    ]
    in_maps = []
    for c in range(8):
        b, g = divmod(c, 4)
        biases = np.ascontiguousarray(
            np.concatenate(
                [
                    bq_t[g],
                    np.concatenate([bq_t[g][64:], bq_t[g][:64]], 0),
                    bk_t[g],
                    np.concatenate([bk_t[g][64:], bk_t[g][:64]], 0),
                    bv[g * HD : (g + 1) * HD, None].astype(np.float32),
                ],
                1,
            )
        )  # (128, 11)
        in_maps.append(
            {
                "xT": xT[b],
                "wqT": np.ascontiguousarray(
                    wq[g * O_G : (g + 1) * O_G].T
                ).astype(BF16),
                "wkvT": np.ascontiguousarray(
                    np.concatenate(
                        [
                            wk[g * HD : (g + 1) * HD].T,
                            wv[g * HD : (g + 1) * HD].T,
                        ],
                        1,
                    )
                ).astype(BF16),
                "wpT": np.ascontiguousarray(
                    wproj[:, g * O_G : (g + 1) * O_G].T
                ).astype(BF16),
                "biases": biases,
                "ropes": ropes,
                "masks": masks,
            }
        )

    res = run_bass_kernel_spmd(nc, in_maps, core_ids=list(range(8)))
    parts = [res.results[c]["out"].astype(np.float32) for c in range(8)]
    out = np.stack(
        [
            parts[0] + parts[1] + parts[2] + parts[3],
            parts[4] + parts[5] + parts[6] + parts[7],
        ]
    ).astype(np.float32)
    return out

